# revision 11
# baseline (speedup 1.0000x reference)
"""Causal self-attention (B=4, T=2048, C=1024, H=16, D=64) on 8 trn2 cores.

Sharding: hybrid batch x head — core c handles batch g = c//2 and heads
e*8..e*8+7 (e = c%2).  Each core computes qkv for its batch (x batch-sliced,
w_qkv column-sliced to 8 heads), RoPE, causal attention, and a partial output
projection (w_proj row-sliced); the host sums the 2 partial projections per
batch.  vs pure head-parallel this cuts per-core HBM traffic ~4x (x read and
out write are 1 batch instead of 4).

Device layout notes:
  - xT [C, T] pre-tiled on host so each SBUF partition line is one contiguous
    DMA run; all matmul inputs bf16 (fp32 PSUM accumulation).
  - RoPE: q/k features are PERMUTED on the host (pos 2m <- feat m,
    pos 2m+1 <- feat m+32 per 64-block) so rotate_half becomes an
    adjacent-partition swap = one DVE stream_shuffle; the sign and the
    cos/sin pairing fold into host-built cosP/sinP tables.  Scores are
    permutation-invariant; v/y stay unpermuted.
  - Attention in "S^T" layout [j, i] per head-pair (2 heads co-resident on
    the PE array via tile_position row groups); softmax denominator from an
    appended ones-column in V (PV emits sum(exp) as row 64); causal masking
    by multiplying exp tiles with one canonical [j, c] ramp mask (DVE);
    diagonal-crossing chunks trim fully-masked leading columns from the
    scores/PV matmuls (N = 512-delta).
  - Phase pipelining: attention(ti) only needs rows <= (ti+1)*512, so qkv
    matmuls for row-tile ti+1 are interleaved ("fillers") into attention(ti)
    where the PE would otherwise idle waiting on Scalar-engine EXPs.
  - Projection partials written bf16; host sums the 2 partials per batch.
"""
import numpy as np
import ml_dtypes

import concourse.bass as bass
import concourse.mybir as mybir
import concourse.tile as tile

B, T, C = 4, 2048, 1024
H, D = 16, 64
NCORES = 8
HPC = 8                    # heads per core
NB = 2                     # batch groups share a batch via head split
N = T                      # rows per core = one batch
P = 128
FD = 512                   # i-tile / row tile
KO = C // P                # 8 k-chunks for qkv
NT = N // FD               # 4 row tiles
NPAIR = 4                  # head pairs per core
ROPE_BASE = 10000.0

F32 = mybir.dt.float32
F16 = mybir.dt.float16
BF16 = mybir.dt.bfloat16
AF = mybir.ActivationFunctionType

# stream_shuffle mask: swap adjacent partitions within each 32-quadrant
SWAP_ADJ = [i ^ 1 for i in range(32)]


def build_nc():
    nc = bass.Bass()
    # xT[p, nt, ko, f] = x[g, nt*FD+f, ko*128+p]
    xT = nc.dram_tensor("xT", [P, NT, KO, FD], BF16, kind="ExternalInput")
    w_all = nc.dram_tensor("w_all", [C, 3 * FD], BF16, kind="ExternalInput")
    wp = nc.dram_tensor("wp", [P, NPAIR, C], BF16, kind="ExternalInput")
    cosb = nc.dram_tensor("cosb", [P, T], BF16, kind="ExternalInput")
    sinb = nc.dram_tensor("sinb", [P, T], BF16, kind="ExternalInput")
    maskb = nc.dram_tensor("maskb", [P, FD], BF16, kind="ExternalInput")
    out = nc.dram_tensor("out", [N, C], BF16, kind="ExternalOutput")

    w_r = w_all.rearrange("(ko p) m -> p ko m", p=P)        # [128, 8, 1536]

    with tile.TileContext(nc) as tc:
        with tc.tile_pool(name="persist", bufs=1) as persist, \
             tc.tile_pool(name="work", bufs=2) as work, \
             tc.tile_pool(name="psum", bufs=1, space="PSUM") as psum, \
             tc.tile_pool(name="dstage", bufs=1, space="DRAM") as dstage:
            vt_dram = [dstage.tile([P, T], BF16, tag=f"vt{p}", name=f"vt{p}")
                       for p in range(NPAIR)]

            from concourse.library_config import all_libraries
            attn_lib = next(l for l in all_libraries if l.name == "attn")
            nc.gpsimd.load_library(attn_lib)

            # split the weight load per (m, fg) column-slice so the first qkv
            # matmuls start as soon as their slice lands (kills ~25us of
            # startup dead time vs one monolithic 3MB dma)
            w_sb = persist.tile([P, KO, 3 * FD], BF16, tag="w_sb")
            for m in range(3):
                for fg in range(NPAIR):
                    col = m * FD + fg * P
                    nc.sync.dma_start(w_sb[:, :, col:col + P],
                                      w_r[:, :, col:col + P])
            cos_sb = persist.tile([P, T], BF16, tag="cos_sb")
            nc.sync.dma_start(cos_sb, cosb[:, :])
            sin_sb = persist.tile([P, T], BF16, tag="sin_sb")
            nc.sync.dma_start(sin_sb, sinb[:, :])
            mask_sb = persist.tile([P, FD], BF16, tag="mask_sb")
            nc.sync.dma_start(mask_sb, maskb[:, :])
            wp_sb = persist.tile([P, NPAIR, C], BF16, tag="wp_sb")
            nc.sync.dma_start(wp_sb, wp[:, :, :])

            qT = persist.tile([P, NPAIR, N], BF16, tag="qT")
            kT = persist.tile([P, NPAIR, N], BF16, tag="kT")
            yT = persist.tile([P, NPAIR, N], BF16, tag="yT")
            # vsb[h]: PV lhsT [j-part, chunk, D+1], ones in col D
            vsb = [persist.tile([P, T // P, D + 1], BF16, tag=f"vsb{h}",
                                name=f"vsb{h}")
                   for h in range(HPC)]
            for h in range(HPC):
                nc.any.memset(vsb[h][:, :, D:D + 1], 1.0)

            # ---------- emission helpers ----------
            def emit_qkv_tile(nt):
                """qkv matmuls + rope/v evacuation for row-tile nt.
                Returns per-matmul closures for fine-grained interleaving."""
                closures = []
                i0 = nt * FD
                x_sb = work.tile([P, KO, FD], BF16, tag="x_sb",
                                 name=f"x_sb{nt}")
                for ko in range(KO):
                    nc.sync.dma_start(x_sb[:, ko, :], xT[:, nt, ko, :])

                def evac(m, fg, ps):
                    if m == 2:      # v: stage to DRAM for transpose reload
                        v_bf = work.tile([P, FD], BF16, tag="v_bf",
                                         name=f"v_bf{nt}_{fg}")
                        nc.vector.tensor_copy(v_bf[:], ps[:])
                        nc.sync.dma_start(vt_dram[fg][:, i0:i0 + FD], v_bf[:])
                    else:           # q/k: rope in permuted basis
                        dst = qT if m == 0 else kT
                        a = work.tile([P, FD], BF16, tag="rope_a",
                                      name=f"a{nt}_{m}_{fg}")
                        nc.vector.tensor_mul(a[:], ps[:], cos_sb[:, i0:i0 + FD])
                        b = work.tile([P, FD], BF16, tag="rope_b",
                                      name=f"b{nt}_{m}_{fg}")
                        nc.vector.tensor_mul(b[:], ps[:], sin_sb[:, i0:i0 + FD])
                        br = work.tile([P, FD], BF16, tag="rope_br",
                                       name=f"br{nt}_{m}_{fg}")
                        nc.vector.stream_shuffle(br[:], b[:], SWAP_ADJ)
                        nc.vector.tensor_add(dst[:, fg, i0:i0 + FD], a[:], br[:])

                def mk(m, fg, ko, cell):
                    def run():
                        if ko == 0:
                            cell[0] = psum.tile([P, FD], F32, tag="qk", bufs=2,
                                                name=f"qk{nt}_{m}_{fg}")
                        ps = cell[0]
                        col = m * FD + fg * P
                        nc.tensor.matmul(ps, w_sb[:, ko, col:col + P],
                                         x_sb[:, ko, :],
                                         start=(ko == 0), stop=(ko == KO - 1))
                        if ko == KO - 1:
                            evac(m, fg, ps)
                    return run

                for fg in range(NPAIR):
                    for m in range(3):
                        cell = [None]
                        for ko in range(KO):
                            closures.append(mk(m, fg, ko, cell))
                return closures

            def emit_vload(nt):
                """Transpose-reload this row-tile's staged v into vsb."""
                i0 = nt * FD
                for pr in range(NPAIR):
                    vc = work.tile([P, FD // P, P], BF16, tag="vc",
                                   name=f"vc{nt}_{pr}")
                    nc.sync.dma_start_transpose(vc[:, :, :],
                                                vt_dram[pr][:, i0:i0 + FD])
                    c0 = i0 // P
                    nc.vector.tensor_copy(
                        vsb[2 * pr][:, c0:c0 + FD // P, 0:D], vc[:, :, 0:D])
                    nc.vector.tensor_copy(
                        vsb[2 * pr + 1][:, c0:c0 + FD // P, 0:D], vc[:, :, D:P])

            fillers = []

            def pump(k):
                for _ in range(min(k, len(fillers))):
                    fillers.pop(0)()

            def emit_attention(ti):
                i0 = ti * FD
                njc = (ti + 1) * (FD // P)
                events = NPAIR * njc + 1
                for pr in range(NPAIR):
                    hs = (2 * pr, 2 * pr + 1)
                    py = [psum.tile([D + 1, FD], F32, tag=f"py{hi}",
                                    name=f"py{ti}_{pr}_{hi}")
                          for hi in range(2)]
                    for jb in range(0, njc, 2):
                        nb = min(2, njc - jb)
                        sts = [psum.tile([P, 2, FD], F32, tag="st",
                                         bufs=2, name=f"st{ti}_{pr}_{hi}")
                               for hi in range(2)]
                        deltas = []
                        for c in range(nb):
                            jc = jb + c
                            dlt = jc * P - i0
                            d = dlt if dlt > 0 else 0
                            deltas.append(d)
                            for hi in range(2):
                                r0 = hi * D
                                nc.tensor.matmul(
                                    sts[hi][:, c, d:FD],
                                    kT[r0:r0 + D, pr, jc * P:jc * P + P],
                                    qT[r0:r0 + D, pr, i0 + d:i0 + FD],
                                    start=True, stop=True,
                                    tile_position=(r0, 0))
                        exs = []
                        for hi in range(2):
                            ex = work.tile([P, 2, FD], BF16, tag=f"ex{hi}",
                                           name=f"ex{ti}_{pr}_{hi}")
                            if all(d == 0 for d in deltas):
                                nc.scalar.activation(
                                    ex[:, 0:nb, :], sts[hi][:, 0:nb, :],
                                    AF.Exp, scale=float(D) ** -0.5)
                            else:
                                for c in range(nb):
                                    d = deltas[c]
                                    nc.scalar.activation(
                                        ex[:, c, d:FD], sts[hi][:, c, d:FD],
                                        AF.Exp, scale=float(D) ** -0.5)
                            exs.append(ex)
                        for c in range(nb):
                            d = deltas[c]
                            jc = jb + c
                            if jc * P - i0 > -P:  # diagonal-crossing chunk
                                for hi in range(2):
                                    nc.vector.tensor_mul(
                                        exs[hi][:, c, d:FD],
                                        exs[hi][:, c, d:FD],
                                        mask_sb[:, 0:FD - d])
                        for c in range(nb):
                            jc = jb + c
                            d = deltas[c]
                            for hi in range(2):
                                nc.tensor.matmul(
                                    py[hi][:, d:FD], vsb[hs[hi]][:, jc, :],
                                    exs[hi][:, c, d:FD],
                                    start=(jc == 0), stop=(jc == njc - 1))
                            pump(-(-len(fillers) // events))
                            events -= 1
                    # normalize: yT = pv / l (l = row D of py)
                    for hi in range(2):
                        lrow = work.tile([P, FD], F32, tag="lrow",
                                         name=f"lrow{ti}_{pr}_{hi}")
                        nc.vector.tensor_copy(lrow[0:1, :], py[hi][D:D + 1, :])
                        rc = work.tile([P, FD], F32, tag="rc",
                                       name=f"rc{ti}_{pr}_{hi}")
                        nc.vector.reciprocal_approx_fast(rc[0:1, :], lrow[0:1, :])
                        rb = work.tile([D, FD], F32, tag="rb",
                                       name=f"rb{ti}_{pr}_{hi}")
                        nc.gpsimd.partition_broadcast(rb[:, :], rc[0:1, :])
                        nc.vector.tensor_mul(
                            yT[hi * D:(hi + 1) * D, pr, i0:i0 + FD],
                            py[hi][0:D, :], rb[:, :])
                pump(len(fillers))

            def emit_proj(ti):
                """Per-(row-chunk, col-half) closures: 4 accum matmuls +
                psum evac; the out-row DMA rides on the second col-half."""
                closures = []

                def mk(mtl, nh, cell):
                    def run():
                        r0 = ti * FD + mtl * P
                        if nh == 0:
                            cell[0] = work.tile([P, C], BF16, tag="o_sb",
                                                name=f"o_sb{ti}_{mtl}")
                        o_sb = cell[0]
                        po = psum.tile([P, FD], F32, tag="qk", bufs=2,
                                       name=f"po{ti}_{mtl}_{nh}")
                        for fg in range(NPAIR):
                            nc.tensor.matmul(
                                po, yT[:, fg, r0:r0 + P],
                                wp_sb[:, fg, nh * FD:(nh + 1) * FD],
                                start=(fg == 0), stop=(fg == NPAIR - 1))
                        nc.vector.tensor_copy(
                            o_sb[:, nh * FD:(nh + 1) * FD], po[:])
                        if nh == (C // FD) - 1:
                            nc.sync.dma_start(out[r0:r0 + P, :], o_sb[:])
                    return run

                for mtl in range(FD // P):
                    cell = [None]
                    for nh in range(C // FD):
                        closures.append(mk(mtl, nh, cell))
                return closures

            # ---------- main schedule ----------
            for cl in emit_qkv_tile(0):
                cl()
            for ti in range(NT):
                emit_vload(ti)
                if ti + 1 < NT:
                    fillers.extend(emit_qkv_tile(ti + 1))
                emit_attention(ti)
                # proj(ti) closures get pumped into attention(ti+1);
                # the last tile's projection runs inline.
                if ti + 1 < NT:
                    fillers.extend(emit_proj(ti))
                else:
                    for cl in emit_proj(ti):
                        cl()
    return nc


def split_multi_waits(nc):
    """walrus encodes only ONE sem wait per TPB instruction and does not
    auto-split. Hoist extra waits onto same-engine nops."""
    for blk in nc.main_func.blocks:
        new_insts = []
        for inst in blk.instructions:
            si = inst.sync_info
            if si is not None and si.on_wait and len(si.on_wait) > 1:
                for w in si.on_wait[:-1]:
                    nop = mybir.InstNoOp(
                        name=nc.get_next_instruction_name(), ins=[], outs=[])
                    nop.engine = inst.engine
                    nop.sync_info = mybir.SyncInfo(on_wait=[w], on_update=[])
                    nc.register_instruction(nop)
                    new_insts.append(nop)
                si.on_wait = si.on_wait[-1:]
            new_insts.append(inst)
        blk.instructions[:] = new_insts


def _feat_perm():
    """Within each 64-feature head block: pos 2m <- feat m, 2m+1 <- feat m+32."""
    p = np.empty(D, np.int64)
    half = D // 2
    p[0::2] = np.arange(half)
    p[1::2] = np.arange(half) + half
    return p


def _rope_tables():
    inv_freq = 1.0 / (ROPE_BASE ** (np.arange(0, D, 2, dtype=np.float32) / D))
    t = np.arange(T, dtype=np.float32)
    freqs = np.outer(inv_freq, t)                       # [32, T]
    cos = np.cos(freqs)
    sin = np.sin(freqs)
    cosP = np.empty((D, T), np.float32)                 # permuted-basis tables
    sinP = np.empty((D, T), np.float32)
    cosP[0::2] = cos
    cosP[1::2] = cos
    sinP[0::2] = sin
    sinP[1::2] = -sin
    cos2 = np.concatenate([cosP, cosP], 0)              # [128, T]
    sin2 = np.concatenate([sinP, sinP], 0)
    return cos2.astype(ml_dtypes.bfloat16), sin2.astype(ml_dtypes.bfloat16)


def make_in_maps(x, w_qkv, w_proj):
    x = np.asarray(x, np.float32)
    w_qkv = np.asarray(w_qkv, np.float32)
    w_proj = np.asarray(w_proj, np.float32)
    cos2, sin2 = _rope_tables()
    mask = (np.arange(FD)[None, :] >= np.arange(P)[:, None])
    mask = mask.astype(ml_dtypes.bfloat16)
    perm = _feat_perm()

    in_maps = []
    for c in range(NCORES):
        g, e = c // NB, c % NB
        # xT[p, nt, ko, f] = x[g, nt*FD+f, ko*128+p]
        xT = np.ascontiguousarray(
            x[g].reshape(NT, FD, KO, P).transpose(3, 0, 2, 1)
        ).astype(ml_dtypes.bfloat16)
        cols = slice(e * HPC * D, (e + 1) * HPC * D)
        wq = w_qkv[:, 0 * C:1 * C][:, cols].reshape(C, HPC, D)[:, :, perm]
        wk = w_qkv[:, 1 * C:2 * C][:, cols].reshape(C, HPC, D)[:, :, perm]
        wv = w_qkv[:, 2 * C:3 * C][:, cols]
        w_all = np.concatenate(
            [wq.reshape(C, HPC * D), wk.reshape(C, HPC * D), wv], axis=1)
        # wp[p, pair, n] = w_proj[e*512 + pair*128 + p, n]
        wp_t = np.ascontiguousarray(
            w_proj[cols, :].reshape(NPAIR, P, C).transpose(1, 0, 2))
        in_maps.append({
            "xT": xT,
            "w_all": np.ascontiguousarray(w_all).astype(ml_dtypes.bfloat16),
            "wp": wp_t.astype(ml_dtypes.bfloat16),
            "cosb": cos2,
            "sinb": sin2,
            "maskb": mask,
        })
    return in_maps


_NC_CACHE = {}


def kernel(x, w_qkv, w_proj):
    from concourse.bass_utils import run_bass_kernel_spmd
    if "nc" not in _NC_CACHE:
        nc0 = build_nc()
        from concourse.library_overlay import lower_extended_insts
        lower_extended_insts(nc0)
        split_multi_waits(nc0)
        _NC_CACHE["nc"] = nc0
    nc = _NC_CACHE["nc"]
    in_maps = make_in_maps(x, w_qkv, w_proj)
    res = run_bass_kernel_spmd(nc, in_maps, list(range(NCORES)))
    acc = np.zeros((B, N, C), np.float64)
    for c, r in enumerate(res.results):
        acc[c // NB] += r["out"].astype(np.float64)
    return acc.astype(np.float32)


# revision 19
# speedup vs baseline: 1.0288x; 1.0288x over previous
"""Causal self-attention (B=4, T=2048, C=1024, H=16, D=64) on 8 trn2 cores.

Sharding: hybrid batch x head — core c handles batch g = c//2 and heads
e*8..e*8+7 (e = c%2).  Each core computes qkv for its batch (x batch-sliced,
w_qkv column-sliced to 8 heads), RoPE, causal attention, and a partial output
projection (w_proj row-sliced); the host sums the 2 partial projections per
batch.  vs pure head-parallel this cuts per-core HBM traffic ~4x (x read and
out write are 1 batch instead of 4).

Device layout notes:
  - xT [C, T] pre-tiled on host so each SBUF partition line is one contiguous
    DMA run; all matmul inputs bf16 (fp32 PSUM accumulation).
  - RoPE: q/k features are PERMUTED on the host (pos 2m <- feat m,
    pos 2m+1 <- feat m+32 per 64-block) so rotate_half becomes an
    adjacent-partition swap = one DVE stream_shuffle; the sign and the
    cos/sin pairing fold into host-built cosP/sinP tables.  Scores are
    permutation-invariant; v/y stay unpermuted.
  - Attention in "S^T" layout [j, i] per head-pair (2 heads co-resident on
    the PE array via tile_position row groups); softmax denominator from an
    appended ones-column in V (PV emits sum(exp) as row 64); causal masking
    by multiplying exp tiles with one canonical [j, c] ramp mask (DVE);
    diagonal-crossing chunks trim fully-masked leading columns from the
    scores/PV matmuls (N = 512-delta).
  - Phase pipelining: attention(ti) only needs rows <= (ti+1)*512, so qkv
    matmuls for row-tile ti+1 are interleaved ("fillers") into attention(ti)
    where the PE would otherwise idle waiting on Scalar-engine EXPs.
  - Projection partials written bf16; host sums the 2 partials per batch.
"""
import numpy as np
import ml_dtypes

import concourse.bass as bass
import concourse.mybir as mybir
import concourse.tile as tile

B, T, C = 4, 2048, 1024
H, D = 16, 64
NCORES = 8
HPC = 8                    # heads per core
NB = 2                     # batch groups share a batch via head split
N = T                      # rows per core = one batch
P = 128
FD = 512                   # i-tile / row tile
KO = C // P                # 8 k-chunks for qkv
NT = N // FD               # 4 row tiles
NPAIR = 4                  # head pairs per core
ROPE_BASE = 10000.0

F32 = mybir.dt.float32
F16 = mybir.dt.float16
BF16 = mybir.dt.bfloat16
AF = mybir.ActivationFunctionType

# stream_shuffle mask: swap adjacent partitions within each 32-quadrant
SWAP_ADJ = [i ^ 1 for i in range(32)]


def build_nc():
    nc = bass.Bass()
    # xT[p, nt, ko, f] = x[g, nt*FD+f, ko*128+p]
    xT = nc.dram_tensor("xT", [P, NT, KO, FD], BF16, kind="ExternalInput")
    # w pre-sliced on host: w_all[s, p, ko, c] = w[ko*128+p, s*128+c], s=m*4+fg
    w_all = nc.dram_tensor("w_all", [12, P, KO, P], BF16, kind="ExternalInput")
    wp = nc.dram_tensor("wp", [P, NPAIR, C], BF16, kind="ExternalInput")
    cosb = nc.dram_tensor("cosb", [P, T], BF16, kind="ExternalInput")
    sinb = nc.dram_tensor("sinb", [P, T], BF16, kind="ExternalInput")
    maskb = nc.dram_tensor("maskb", [P, FD], BF16, kind="ExternalInput")
    out = nc.dram_tensor("out", [N, C], BF16, kind="ExternalOutput")

    with tile.TileContext(nc) as tc:
        with tc.tile_pool(name="persist", bufs=1) as persist, \
             tc.tile_pool(name="work", bufs=2) as work, \
             tc.tile_pool(name="psum", bufs=1, space="PSUM") as psum, \
             tc.tile_pool(name="dstage", bufs=1, space="DRAM") as dstage:
            vt_dram = [dstage.tile([P, T], BF16, tag=f"vt{p}", name=f"vt{p}")
                       for p in range(NPAIR)]

            from concourse.library_config import all_libraries
            attn_lib = next(l for l in all_libraries if l.name == "attn")
            nc.gpsimd.load_library(attn_lib)

            # Startup-latency-ordered loads: the first qkv group's weight
            # slice and x tile land first, so matmuls start ~13us in instead
            # of waiting out one monolithic load train (~35us).
            w_sb = persist.tile([P, 12, KO, P], BF16, tag="w_sb")
            cos_sb = persist.tile([P, T], BF16, tag="cos_sb")
            sin_sb = persist.tile([P, T], BF16, tag="sin_sb")
            mask_sb = persist.tile([P, FD], BF16, tag="mask_sb")
            wp_sb = persist.tile([P, NPAIR, C], BF16, tag="wp_sb")

            nc.sync.dma_start(w_sb[:, 0], w_all[0])
            x_sb0 = work.tile([P, KO, FD], BF16, tag="x_sb", name="x_sb0")
            for ko in range(KO):
                nc.sync.dma_start(x_sb0[:, ko, :], xT[:, 0, ko, :])
            nc.sync.dma_start(cos_sb, cosb[:, :])
            nc.sync.dma_start(sin_sb, sinb[:, :])
            # remaining w slices in first-use order (qkv groups go fg-major)
            for s in [4, 8, 1, 5, 9, 2, 6, 10, 3, 7, 11]:
                nc.sync.dma_start(w_sb[:, s], w_all[s])
            nc.sync.dma_start(mask_sb, maskb[:, :])
            nc.sync.dma_start(wp_sb, wp[:, :, :])

            qT = persist.tile([P, NPAIR, N], BF16, tag="qT")
            kT = persist.tile([P, NPAIR, N], BF16, tag="kT")
            yT = persist.tile([P, NPAIR, N], BF16, tag="yT")
            # vsb[h]: PV lhsT [j-part, chunk, D+1], ones in col D
            vsb = [persist.tile([P, T // P, D + 1], BF16, tag=f"vsb{h}",
                                name=f"vsb{h}")
                   for h in range(HPC)]
            for h in range(HPC):
                nc.any.memset(vsb[h][:, :, D:D + 1], 1.0)

            # ---------- emission helpers ----------
            def emit_qkv_tile(nt, x_pre=None):
                """qkv matmuls + rope/v evacuation for row-tile nt.
                Returns per-matmul closures for fine-grained interleaving."""
                closures = []
                i0 = nt * FD
                if x_pre is not None:
                    x_sb = x_pre
                else:
                    x_sb = work.tile([P, KO, FD], BF16, tag="x_sb",
                                     name=f"x_sb{nt}")
                    for ko in range(KO):
                        nc.sync.dma_start(x_sb[:, ko, :], xT[:, nt, ko, :])

                def evac(m, fg, ps):
                    if m == 2:      # v: stage to DRAM for transpose reload
                        v_bf = work.tile([P, FD], BF16, tag="v_bf",
                                         name=f"v_bf{nt}_{fg}")
                        nc.vector.tensor_copy(v_bf[:], ps[:])
                        nc.sync.dma_start(vt_dram[fg][:, i0:i0 + FD], v_bf[:])
                    else:           # q/k: rope in permuted basis
                        dst = qT if m == 0 else kT
                        a = work.tile([P, FD], BF16, tag="rope_a",
                                      name=f"a{nt}_{m}_{fg}")
                        nc.vector.tensor_mul(a[:], ps[:], cos_sb[:, i0:i0 + FD])
                        b = work.tile([P, FD], BF16, tag="rope_b",
                                      name=f"b{nt}_{m}_{fg}")
                        nc.vector.tensor_mul(b[:], ps[:], sin_sb[:, i0:i0 + FD])
                        br = work.tile([P, FD], BF16, tag="rope_br",
                                       name=f"br{nt}_{m}_{fg}")
                        nc.vector.stream_shuffle(br[:], b[:], SWAP_ADJ)
                        nc.vector.tensor_add(dst[:, fg, i0:i0 + FD], a[:], br[:])

                def mk(m, fg, ko, cell):
                    def run():
                        if ko == 0:
                            cell[0] = psum.tile([P, FD], F32, tag="qk", bufs=2,
                                                name=f"qk{nt}_{m}_{fg}")
                        ps = cell[0]
                        nc.tensor.matmul(ps, w_sb[:, m * NPAIR + fg, ko, :],
                                         x_sb[:, ko, :],
                                         start=(ko == 0), stop=(ko == KO - 1))
                        if ko == KO - 1:
                            evac(m, fg, ps)
                    return run

                for fg in range(NPAIR):
                    for m in range(3):
                        cell = [None]
                        for ko in range(KO):
                            closures.append(mk(m, fg, ko, cell))
                return closures

            def emit_vload(nt):
                """Transpose-reload this row-tile's staged v into vsb."""
                i0 = nt * FD
                for pr in range(NPAIR):
                    vc = work.tile([P, FD // P, P], BF16, tag="vc",
                                   name=f"vc{nt}_{pr}")
                    nc.sync.dma_start_transpose(vc[:, :, :],
                                                vt_dram[pr][:, i0:i0 + FD])
                    c0 = i0 // P
                    nc.vector.tensor_copy(
                        vsb[2 * pr][:, c0:c0 + FD // P, 0:D], vc[:, :, 0:D])
                    nc.vector.tensor_copy(
                        vsb[2 * pr + 1][:, c0:c0 + FD // P, 0:D], vc[:, :, D:P])

            fillers = []

            def pump(k):
                for _ in range(min(k, len(fillers))):
                    fillers.pop(0)()

            def emit_attention(ti):
                i0 = ti * FD
                njc = (ti + 1) * (FD // P)
                events = NPAIR * njc + 1
                for pr in range(NPAIR):
                    hs = (2 * pr, 2 * pr + 1)
                    py = [psum.tile([D + 1, FD], F32, tag=f"py{hi}",
                                    name=f"py{ti}_{pr}_{hi}")
                          for hi in range(2)]
                    for jb in range(0, njc, 2):
                        nb = min(2, njc - jb)
                        sts = [psum.tile([P, 2, FD], F32, tag="st",
                                         bufs=2, name=f"st{ti}_{pr}_{hi}")
                               for hi in range(2)]
                        deltas = []
                        for c in range(nb):
                            jc = jb + c
                            dlt = jc * P - i0
                            d = dlt if dlt > 0 else 0
                            deltas.append(d)
                            for hi in range(2):
                                r0 = hi * D
                                nc.tensor.matmul(
                                    sts[hi][:, c, d:FD],
                                    kT[r0:r0 + D, pr, jc * P:jc * P + P],
                                    qT[r0:r0 + D, pr, i0 + d:i0 + FD],
                                    start=True, stop=True,
                                    tile_position=(r0, 0))
                        exs = []
                        for hi in range(2):
                            ex = work.tile([P, 2, FD], BF16, tag=f"ex{hi}",
                                           name=f"ex{ti}_{pr}_{hi}")
                            if all(d == 0 for d in deltas):
                                nc.scalar.activation(
                                    ex[:, 0:nb, :], sts[hi][:, 0:nb, :],
                                    AF.Exp, scale=float(D) ** -0.5)
                            else:
                                for c in range(nb):
                                    d = deltas[c]
                                    nc.scalar.activation(
                                        ex[:, c, d:FD], sts[hi][:, c, d:FD],
                                        AF.Exp, scale=float(D) ** -0.5)
                            exs.append(ex)
                        for c in range(nb):
                            d = deltas[c]
                            jc = jb + c
                            if jc * P - i0 > -P:  # diagonal-crossing chunk
                                for hi in range(2):
                                    nc.vector.tensor_mul(
                                        exs[hi][:, c, d:FD],
                                        exs[hi][:, c, d:FD],
                                        mask_sb[:, 0:FD - d])
                        for c in range(nb):
                            jc = jb + c
                            d = deltas[c]
                            for hi in range(2):
                                nc.tensor.matmul(
                                    py[hi][:, d:FD], vsb[hs[hi]][:, jc, :],
                                    exs[hi][:, c, d:FD],
                                    start=(jc == 0), stop=(jc == njc - 1))
                            pump(-(-len(fillers) // events))
                            events -= 1
                    # normalize: yT = pv / l (l = row D of py)
                    for hi in range(2):
                        lrow = work.tile([P, FD], F32, tag="lrow",
                                         name=f"lrow{ti}_{pr}_{hi}")
                        nc.vector.tensor_copy(lrow[0:1, :], py[hi][D:D + 1, :])
                        rc = work.tile([P, FD], F32, tag="rc",
                                       name=f"rc{ti}_{pr}_{hi}")
                        nc.vector.reciprocal_approx_fast(rc[0:1, :], lrow[0:1, :])
                        rb = work.tile([D, FD], F32, tag="rb",
                                       name=f"rb{ti}_{pr}_{hi}")
                        nc.gpsimd.partition_broadcast(rb[:, :], rc[0:1, :])
                        nc.vector.tensor_mul(
                            yT[hi * D:(hi + 1) * D, pr, i0:i0 + FD],
                            py[hi][0:D, :], rb[:, :])
                pump(len(fillers))

            def emit_proj(ti):
                """Per-(row-chunk, col-half) closures: 4 accum matmuls +
                psum evac; the out-row DMA rides on the second col-half."""
                closures = []

                def mk(mtl, nh, cell):
                    def run():
                        r0 = ti * FD + mtl * P
                        if nh == 0:
                            cell[0] = work.tile([P, C], BF16, tag="o_sb",
                                                name=f"o_sb{ti}_{mtl}")
                        o_sb = cell[0]
                        po = psum.tile([P, FD], F32, tag="qk", bufs=2,
                                       name=f"po{ti}_{mtl}_{nh}")
                        for fg in range(NPAIR):
                            nc.tensor.matmul(
                                po, yT[:, fg, r0:r0 + P],
                                wp_sb[:, fg, nh * FD:(nh + 1) * FD],
                                start=(fg == 0), stop=(fg == NPAIR - 1))
                        nc.vector.tensor_copy(
                            o_sb[:, nh * FD:(nh + 1) * FD], po[:])
                        if nh == (C // FD) - 1:
                            nc.sync.dma_start(out[r0:r0 + P, :], o_sb[:])
                    return run

                for mtl in range(FD // P):
                    cell = [None]
                    for nh in range(C // FD):
                        closures.append(mk(mtl, nh, cell))
                return closures

            # ---------- main schedule ----------
            for cl in emit_qkv_tile(0, x_pre=x_sb0):
                cl()
            for ti in range(NT):
                emit_vload(ti)
                if ti + 1 < NT:
                    fillers.extend(emit_qkv_tile(ti + 1))
                emit_attention(ti)
                # proj(ti) closures get pumped into attention(ti+1);
                # the last tile's projection runs inline.
                if ti + 1 < NT:
                    fillers.extend(emit_proj(ti))
                else:
                    for cl in emit_proj(ti):
                        cl()
    return nc


def split_multi_waits(nc):
    """walrus encodes only ONE sem wait per TPB instruction and does not
    auto-split. Hoist extra waits onto same-engine nops."""
    for blk in nc.main_func.blocks:
        new_insts = []
        for inst in blk.instructions:
            si = inst.sync_info
            if si is not None and si.on_wait and len(si.on_wait) > 1:
                for w in si.on_wait[:-1]:
                    nop = mybir.InstNoOp(
                        name=nc.get_next_instruction_name(), ins=[], outs=[])
                    nop.engine = inst.engine
                    nop.sync_info = mybir.SyncInfo(on_wait=[w], on_update=[])
                    nc.register_instruction(nop)
                    new_insts.append(nop)
                si.on_wait = si.on_wait[-1:]
            new_insts.append(inst)
        blk.instructions[:] = new_insts


def _feat_perm():
    """Within each 64-feature head block: pos 2m <- feat m, 2m+1 <- feat m+32."""
    p = np.empty(D, np.int64)
    half = D // 2
    p[0::2] = np.arange(half)
    p[1::2] = np.arange(half) + half
    return p


def _rope_tables():
    inv_freq = 1.0 / (ROPE_BASE ** (np.arange(0, D, 2, dtype=np.float32) / D))
    t = np.arange(T, dtype=np.float32)
    freqs = np.outer(inv_freq, t)                       # [32, T]
    cos = np.cos(freqs)
    sin = np.sin(freqs)
    cosP = np.empty((D, T), np.float32)                 # permuted-basis tables
    sinP = np.empty((D, T), np.float32)
    cosP[0::2] = cos
    cosP[1::2] = cos
    sinP[0::2] = sin
    sinP[1::2] = -sin
    cos2 = np.concatenate([cosP, cosP], 0)              # [128, T]
    sin2 = np.concatenate([sinP, sinP], 0)
    return cos2.astype(ml_dtypes.bfloat16), sin2.astype(ml_dtypes.bfloat16)


def make_in_maps(x, w_qkv, w_proj):
    x = np.asarray(x, np.float32)
    w_qkv = np.asarray(w_qkv, np.float32)
    w_proj = np.asarray(w_proj, np.float32)
    cos2, sin2 = _rope_tables()
    mask = (np.arange(FD)[None, :] >= np.arange(P)[:, None])
    mask = mask.astype(ml_dtypes.bfloat16)
    perm = _feat_perm()

    in_maps = []
    for c in range(NCORES):
        g, e = c // NB, c % NB
        # xT[p, nt, ko, f] = x[g, nt*FD+f, ko*128+p]
        xT = np.ascontiguousarray(
            x[g].reshape(NT, FD, KO, P).transpose(3, 0, 2, 1)
        ).astype(ml_dtypes.bfloat16)
        cols = slice(e * HPC * D, (e + 1) * HPC * D)
        wq = w_qkv[:, 0 * C:1 * C][:, cols].reshape(C, HPC, D)[:, :, perm]
        wk = w_qkv[:, 1 * C:2 * C][:, cols].reshape(C, HPC, D)[:, :, perm]
        wv = w_qkv[:, 2 * C:3 * C][:, cols]
        w_cat = np.concatenate(
            [wq.reshape(C, HPC * D), wk.reshape(C, HPC * D), wv], axis=1)
        # w_all[s, p, ko, c] = w_cat[ko*128+p, s*128+c]
        w_all = np.ascontiguousarray(
            w_cat.reshape(KO, P, 12, P).transpose(2, 1, 0, 3))
        # wp[p, pair, n] = w_proj[e*512 + pair*128 + p, n]
        wp_t = np.ascontiguousarray(
            w_proj[cols, :].reshape(NPAIR, P, C).transpose(1, 0, 2))
        in_maps.append({
            "xT": xT,
            "w_all": w_all.astype(ml_dtypes.bfloat16),
            "wp": wp_t.astype(ml_dtypes.bfloat16),
            "cosb": cos2,
            "sinb": sin2,
            "maskb": mask,
        })
    return in_maps


_NC_CACHE = {}


def kernel(x, w_qkv, w_proj):
    from concourse.bass_utils import run_bass_kernel_spmd
    if "nc" not in _NC_CACHE:
        nc0 = build_nc()
        from concourse.library_overlay import lower_extended_insts
        lower_extended_insts(nc0)
        split_multi_waits(nc0)
        _NC_CACHE["nc"] = nc0
    nc = _NC_CACHE["nc"]
    in_maps = make_in_maps(x, w_qkv, w_proj)
    res = run_bass_kernel_spmd(nc, in_maps, list(range(NCORES)))
    acc = np.zeros((B, N, C), np.float64)
    for c, r in enumerate(res.results):
        acc[c // NB] += r["out"].astype(np.float64)
    return acc.astype(np.float32)
